# revision 1
# baseline (speedup 1.0000x reference)
"""Causal self-attention (dense transformer block) on 8 TRN2 NeuronCores.

Tensor-parallel over heads: 16 heads / 8 cores -> 2 heads per core, both
batch elements on every core. Per core:
  - PE-transpose x tiles to get x^T (contraction layouts)
  - QKV projection in "T layout": q^T/k^T per head [dh, tok], V natural
    [tok, dh] (for both heads), biases for q/k fused into the PSUM eviction
  - causal attention with scores in transposed layout [k, q]: softmax
    numerator via ACT exp (scale folded), denominators via DVE
    partition-chunk accumulation + per-q-block ones-matmuls; unnormalized
    attn output accumulates directly in the [dh, q] layout out_proj needs
  - out_proj per head; 1/rowsum applied as per-partition ACT scale at PSUM
    eviction; heads + folded bias combined on DVE
  - core returns a partial [2, 2048, 2048]; host sums the 8 partials
    (the v-bias term and b_out are folded into a host-precomputed bias)
Matmuls run as float32r (TF32-like, full PE rate at free dim >= 256).
"""
import sys

if "/opt/trn_rl_repo" not in sys.path:
    sys.path.insert(0, "/opt/trn_rl_repo")

import numpy as np

import concourse.bacc as bacc
import concourse.bass as bass
import concourse.mybir as mybir
import concourse.tile as tile
from concourse.bass_utils import run_bass_kernel_spmd

P = 128
B, S, D = 2, 2048, 2048
H, DH = 16, 128
HPC = 2            # heads per core
NCORES = 8
TC = 256           # token chunk for the QKV projection
QC = 512           # q chunk for attention
SCALE = 1.0 / float(np.sqrt(DH))

f32 = mybir.dt.float32
f32r = mybir.dt.float32r
Act = mybir.ActivationFunctionType


def _emit(nc, tc_ctx, aps):
    xty, wqkv, bqk, wout, brep, trilm, ones, out_p = aps
    tc = tc_ctx
    NTB = S // P            # 16 token blocks per batch
    NDC = D // P            # 16 contraction chunks
    NQC = S // QC           # 4 q chunks of 512
    NKB = S // P            # 16 key blocks

    with (
        tc.tile_pool(name="const", bufs=1) as const,
        tc.tile_pool(name="xtp", bufs=2) as xtp,
        tc.tile_pool(name="qk", bufs=1) as qk,
        tc.tile_pool(name="vp", bufs=1) as vp,
        tc.tile_pool(name="pp", bufs=4) as pp,
        tc.tile_pool(name="rs", bufs=1) as rs,
        tc.tile_pool(name="up", bufs=1) as up,
        tc.tile_pool(name="op", bufs=2) as op,
        tc.tile_pool(name="fin", bufs=4) as fin,
        tc.tile_pool(name="ps_w", bufs=4, space="PSUM") as ps_w,
        tc.tile_pool(name="ps_u", bufs=4, space="PSUM") as ps_u,
    ):
        bqk_sb = const.tile([P, 4], f32)
        nc.sync.dma_start(bqk_sb, bqk)
        w_sb = const.tile([P, NDC, 6 * P], f32r)
        nc.sync.dma_start(
            w_sb, wqkv.rearrange("(dc p) c -> p dc c", p=P).bitcast(f32r)
        )
        tril_sb = const.tile([P, P], f32)
        ones_sb = const.tile([P, 1], f32)
        brep_sb = const.tile([P, D], f32)
        wo_sb = const.tile([P, HPC, D], f32r)

        def load_late_consts():
            nc.sync.dma_start(tril_sb, trilm)
            nc.sync.dma_start(ones_sb, ones)
            nc.sync.dma_start(
                wo_sb, wout.rearrange("(h p) c -> p h c", p=P).bitcast(f32r)
            )
            nc.sync.dma_start(brep_sb, brep)

        for b in range(B):
            # ---------------- QKV projection ----------------
            q_sb = [qk.tile([P, S], f32r, tag=f"q{h}", name=f"q{h}") for h in range(HPC)]
            k_sb = [qk.tile([P, S], f32r, tag=f"k{h}", name=f"k{h}") for h in range(HPC)]
            v_sb = vp.tile([P, NTB, HPC * DH], f32r, tag="v", name="v_sb")

            for tci in range(S // TC):
                xt = xtp.tile([P, NDC, TC], f32r, tag="xt", name="xt")
                nc.sync.dma_start(
                    xt,
                    xty[b, :, tci * TC:(tci + 1) * TC]
                    .rearrange("(dc p) t -> p dc t", p=P)
                    .bitcast(f32r),
                )
                # q^T / k^T for both heads: psum [col=128, tok=TC]
                for cb in range(4):
                    psq = ps_w.tile([P, TC], f32, tag="w", name="psq")
                    for dc in range(NDC):
                        nc.tensor.matmul(
                            psq,
                            w_sb[:, dc, cb * P:(cb + 1) * P],
                            xt[:, dc, :],
                            start=(dc == 0),
                            stop=(dc == NDC - 1),
                        )
                    dst = q_sb[cb] if cb < HPC else k_sb[cb - HPC]
                    nc.scalar.activation(
                        dst[:, tci * TC:(tci + 1) * TC],
                        psq,
                        Act.Identity,
                        bias=bqk_sb[:, cb:cb + 1],
                    )
                # V natural for both heads: psum [tok=128, 2*dh]
                for tb in range(TC // P):
                    psv = ps_w.tile([P, HPC * DH], f32, tag="w", name="psv")
                    for dc in range(NDC):
                        nc.tensor.matmul(
                            psv,
                            xt[:, dc, tb * P:(tb + 1) * P],
                            w_sb[:, dc, 4 * P:6 * P],
                            start=(dc == 0),
                            stop=(dc == NDC - 1),
                        )
                    nc.scalar.copy(v_sb[:, tci * (TC // P) + tb, :], psv)

            if b == 0:
                load_late_consts()

            # ---------------- attention per head ----------------
            rinv = []
            u_sb = []
            for h in range(HPC):
                rsum = rs.tile([P, S], f32, tag="rsum", name="rsum")
                us = up.tile([P, S], f32r, tag=f"u{h}", name=f"u{h}")
                psu = [
                    ps_u.tile([P, QC], f32, tag="u", name=f"psu{c}")
                    for c in range(NQC)
                ]
                for kb in range(NKB):
                    c0 = kb // (QC // P)
                    doff = kb * P - c0 * QC
                    for c in range(c0, NQC):
                        psp = ps_w.tile([P, QC], f32, tag="w", name="psp")
                        nc.tensor.matmul(
                            psp,
                            k_sb[h][:, kb * P:(kb + 1) * P],
                            q_sb[h][:, c * QC:(c + 1) * QC],
                            start=True,
                            stop=True,
                        )
                        if c == c0:
                            nc.vector.tensor_add(
                                psp[:, doff:doff + P],
                                psp[:, doff:doff + P],
                                tril_sb,
                            )
                        p_t = pp.tile([P, QC], f32r, tag="p", name="p_t")
                        off = doff if c == c0 else 0
                        nc.scalar.activation(
                            p_t[:, off:], psp[:, off:], Act.Exp, scale=SCALE
                        )
                        if off > 0:
                            nc.vector.memset(p_t[:, :off].bitcast(f32), 0.0)
                        p_f32 = p_t.bitcast(f32)
                        if kb == 0:
                            nc.vector.tensor_copy(
                                out=rsum[:, c * QC:(c + 1) * QC], in_=p_f32
                            )
                        else:
                            nc.vector.tensor_add(
                                rsum[:, c * QC:(c + 1) * QC],
                                rsum[:, c * QC:(c + 1) * QC],
                                p_f32,
                            )
                        nc.tensor.matmul(
                            psu[c],
                            v_sb[:, kb, h * DH:(h + 1) * DH],
                            p_t,
                            start=(kb == 0),
                            stop=(kb == (QC // P) * c + (QC // P) - 1),
                        )
                        if kb == (QC // P) * c + (QC // P) - 1:
                            nc.scalar.copy(us[:, c * QC:(c + 1) * QC], psu[c])
                # denominators: per q-block ones-matmul then reciprocal
                psr = ps_w.tile([P, NTB], f32, tag="w", name="psr")
                for qb in range(NTB):
                    nc.tensor.matmul(
                        psr[:, qb:qb + 1],
                        rsum[:, qb * P:(qb + 1) * P],
                        ones_sb,
                        start=True,
                        stop=True,
                    )
                ri = rs.tile([P, NTB], f32, tag=f"rinv{h}", name=f"rinv{h}")
                nc.vector.reciprocal(ri, psr)
                rinv.append(ri)
                u_sb.append(us)

            # ---------------- out projection ----------------
            for tb in range(NTB):
                for cc in range(D // QC):
                    ps0 = ps_w.tile([P, QC], f32, tag="w", name="ps0")
                    nc.tensor.matmul(
                        ps0,
                        u_sb[0][:, tb * P:(tb + 1) * P],
                        wo_sb[:, 0, cc * QC:(cc + 1) * QC],
                        start=True,
                        stop=True,
                    )
                    ps1 = ps_w.tile([P, QC], f32, tag="w", name="ps1")
                    nc.tensor.matmul(
                        ps1,
                        u_sb[1][:, tb * P:(tb + 1) * P],
                        wo_sb[:, 1, cc * QC:(cc + 1) * QC],
                        start=True,
                        stop=True,
                    )
                    f_t = fin.tile([P, QC], f32, tag="fin", name="f_t")
                    nc.scalar.activation(
                        f_t, ps0, Act.Copy, scale=rinv[0][:, tb:tb + 1]
                    )
                    o1 = op.tile([P, QC], f32, tag="o1", name="o1")
                    nc.scalar.activation(
                        o1, ps1, Act.Copy, scale=rinv[1][:, tb:tb + 1]
                    )
                    nc.vector.tensor_add(f_t, f_t, o1)
                    nc.vector.tensor_add(
                        f_t, f_t, brep_sb[:, cc * QC:(cc + 1) * QC]
                    )
                    nc.sync.dma_start(
                        out_p[b, tb * P:(tb + 1) * P, cc * QC:(cc + 1) * QC],
                        f_t,
                    )


_CACHE = {}


def _build():
    if "nc" in _CACHE:
        return _CACHE["nc"]
    nc = bacc.Bacc("TRN2", target_bir_lowering=False, debug=False)
    xty = nc.dram_tensor("xty", [B, D, S], f32, kind="ExternalInput").ap()
    wqkv = nc.dram_tensor("wqkv", [D, 6 * P], f32, kind="ExternalInput").ap()
    bqk = nc.dram_tensor("bqk", [P, 4], f32, kind="ExternalInput").ap()
    wout = nc.dram_tensor("wout", [HPC * DH, D], f32, kind="ExternalInput").ap()
    brep = nc.dram_tensor("brep", [P, D], f32, kind="ExternalInput").ap()
    trilm = nc.dram_tensor("trilm", [P, P], f32, kind="ExternalInput").ap()
    ones = nc.dram_tensor("ones", [P, 1], f32, kind="ExternalInput").ap()
    out_p = nc.dram_tensor("out_p", [B, S, D], f32, kind="ExternalOutput").ap()
    with tile.TileContext(nc) as tctx:
        _emit(nc, tctx, (xty, wqkv, bqk, wout, brep, trilm, ones, out_p))
    nc.compile()
    _CACHE["nc"] = nc
    return nc


def _in_maps(x, W_qkv, b_qkv, W_out, b_out):
    trilm = np.where(
        np.arange(P)[None, :] >= np.arange(P)[:, None], 0.0, -1e9
    ).astype(np.float32)
    ones = np.ones((P, 1), dtype=np.float32)
    xty = np.ascontiguousarray(x.transpose(0, 2, 1))
    maps = []
    for core in range(NCORES):
        h0 = core * HPC
        cols = []
        for off in (0, D, 2 * D):  # q, k, v column groups of W_qkv
            for h in range(h0, h0 + HPC):
                cols.append((off + h * DH, off + (h + 1) * DH))
        wqkv_c = np.concatenate(
            [W_qkv[:, a:b_] for a, b_ in cols], axis=1
        ).astype(np.float32)
        bqk_c = np.stack(
            [b_qkv[a:b_] for a, b_ in cols[:4]], axis=1
        ).astype(np.float32)  # [128, 4]
        bv_c = np.concatenate(
            [b_qkv[a:b_] for a, b_ in cols[4:]]
        ).astype(np.float32)  # [256]
        wout_c = W_out[h0 * DH:(h0 + HPC) * DH, :].astype(np.float32)
        bias_fold = (b_out / NCORES + bv_c @ wout_c).astype(np.float32)
        brep_c = np.broadcast_to(bias_fold, (P, D)).copy()
        maps.append({
            "xty": xty,
            "wqkv": np.ascontiguousarray(wqkv_c),
            "bqk": np.ascontiguousarray(bqk_c),
            "wout": np.ascontiguousarray(wout_c),
            "brep": brep_c,
            "trilm": trilm,
            "ones": ones,
        })
    return maps


def kernel(x, W_qkv, b_qkv, W_out, b_out, _trace=False, _trace_kwargs=None):
    x = np.asarray(x, dtype=np.float32)
    W_qkv = np.asarray(W_qkv, dtype=np.float32)
    b_qkv = np.asarray(b_qkv, dtype=np.float32)
    W_out = np.asarray(W_out, dtype=np.float32)
    b_out = np.asarray(b_out, dtype=np.float32)

    nc = _build()
    maps = _in_maps(x, W_qkv, b_qkv, W_out, b_out)
    res = run_bass_kernel_spmd(
        nc, maps, core_ids=list(range(NCORES)), trace=_trace,
        **(_trace_kwargs or {}),
    )
    out = res.results[0]["out_p"]
    for c in range(1, NCORES):
        out = out + res.results[c]["out_p"]
    if _trace:
        _CACHE["last_results"] = res
    return out.astype(np.float32)



# revision 8
# speedup vs baseline: 1.0613x; 1.0613x over previous
"""Causal self-attention (dense transformer block) on 8 TRN2 NeuronCores.

Tensor-parallel over heads: 16 heads / 8 cores -> 2 heads per core, both
batch elements on every core. Per core:
  - PE-transpose x tiles to get x^T (contraction layouts)
  - QKV projection in "T layout": q^T/k^T per head [dh, tok], V natural
    [tok, dh] (for both heads), biases for q/k fused into the PSUM eviction
  - causal attention with scores in transposed layout [k, q]: softmax
    numerator via ACT exp (scale folded), denominators via DVE
    partition-chunk accumulation + per-q-block ones-matmuls; unnormalized
    attn output accumulates directly in the [dh, q] layout out_proj needs
  - out_proj per head; 1/rowsum applied as per-partition ACT scale at PSUM
    eviction; heads + folded bias combined on DVE
  - core returns a partial [2, 2048, 2048]; host sums the 8 partials
    (the v-bias term and b_out are folded into a host-precomputed bias)
Matmuls run as float32r (TF32-like, full PE rate at free dim >= 256).
"""
import sys

if "/opt/trn_rl_repo" not in sys.path:
    sys.path.insert(0, "/opt/trn_rl_repo")

import numpy as np

import concourse.bacc as bacc
import concourse.bass as bass
import concourse.mybir as mybir
import concourse.tile as tile
from concourse.bass_utils import run_bass_kernel_spmd

P = 128
B, S, D = 2, 2048, 2048
H, DH = 16, 128
HPC = 2            # heads per core
NCORES = 8
TC = 256           # token chunk for the QKV projection
QC = 512           # q chunk for attention
SCALE = 1.0 / float(np.sqrt(DH))

f32 = mybir.dt.float32
f32r = mybir.dt.float32r
bf16 = mybir.dt.bfloat16
Act = mybir.ActivationFunctionType


def _emit(nc, tc_ctx, aps):
    xty, wqkv, bqk, wout, brep, trilm, ones, out_p = aps
    tc = tc_ctx
    NTB = S // P            # 16 token blocks per batch
    NDC = D // P            # 16 contraction chunks
    NQC = S // QC           # 4 q chunks of 512
    NKB = S // P            # 16 key blocks

    with (
        tc.tile_pool(name="const", bufs=1) as const,
        tc.tile_pool(name="xtp", bufs=2) as xtp,
        tc.tile_pool(name="qk", bufs=1) as qk,
        tc.tile_pool(name="vp", bufs=1) as vp,
        tc.tile_pool(name="pp", bufs=4) as pp,
        tc.tile_pool(name="rs", bufs=1) as rs,
        tc.tile_pool(name="up", bufs=1) as up,
        tc.tile_pool(name="op", bufs=2) as op,
        tc.tile_pool(name="fin", bufs=4) as fin,
        tc.tile_pool(name="ps_w", bufs=4, space="PSUM") as ps_w,
        tc.tile_pool(name="ps_u", bufs=4, space="PSUM") as ps_u,
    ):
        bqk_sb = const.tile([P, 4], f32)
        nc.sync.dma_start(bqk_sb, bqk)
        w_sb = const.tile([P, NDC, 6 * P], bf16)
        nc.sync.dma_start(
            w_sb, wqkv.rearrange("(dc p) c -> p dc c", p=P)
        )
        tril_sb = const.tile([P, P], f32)
        ones_sb = const.tile([P, 1], f32)
        brep_sb = const.tile([P, D], f32)
        wo_sb = const.tile([P, HPC, D], bf16)

        def load_late_consts():
            nc.sync.dma_start(tril_sb, trilm)
            nc.sync.dma_start(ones_sb, ones)
            nc.sync.dma_start(
                wo_sb, wout.rearrange("(h p) c -> p h c", p=P)
            )
            nc.sync.dma_start(brep_sb, brep)

        for b in range(B):
            # ---------------- QKV projection ----------------
            q_sb = [qk.tile([P, S], bf16, tag=f"q{h}", name=f"q{h}") for h in range(HPC)]
            k_sb = [qk.tile([P, S], bf16, tag=f"k{h}", name=f"k{h}") for h in range(HPC)]
            v_sb = vp.tile([P, NTB, HPC * DH], bf16, tag="v", name="v_sb")

            for tci in range(S // TC):
                xt = xtp.tile([P, NDC, TC], bf16, tag="xt", name="xt")
                nc.sync.dma_start(
                    xt,
                    xty[b, :, tci * TC:(tci + 1) * TC]
                    .rearrange("(dc p) t -> p dc t", p=P),
                )
                # q^T / k^T for both heads: psum [col=128, tok=TC]
                for cb in range(4):
                    psq = ps_w.tile([P, TC], f32, tag="w", name="psq")
                    for dc in range(NDC):
                        nc.tensor.matmul(
                            psq,
                            w_sb[:, dc, cb * P:(cb + 1) * P],
                            xt[:, dc, :],
                            start=(dc == 0),
                            stop=(dc == NDC - 1),
                        )
                    dst = q_sb[cb] if cb < HPC else k_sb[cb - HPC]
                    nc.scalar.activation(
                        dst[:, tci * TC:(tci + 1) * TC],
                        psq,
                        Act.Identity,
                        bias=bqk_sb[:, cb:cb + 1],
                    )
                # V natural for both heads: psum [tok=128, 2*dh]
                for tb in range(TC // P):
                    psv = ps_w.tile([P, HPC * DH], f32, tag="w", name="psv")
                    for dc in range(NDC):
                        nc.tensor.matmul(
                            psv,
                            xt[:, dc, tb * P:(tb + 1) * P],
                            w_sb[:, dc, 4 * P:6 * P],
                            start=(dc == 0),
                            stop=(dc == NDC - 1),
                        )
                    nc.scalar.copy(v_sb[:, tci * (TC // P) + tb, :], psv)

            if b == 0:
                load_late_consts()

            # ---------------- attention per head ----------------
            rinv = []
            u_sb = []
            for h in range(HPC):
                rsum = rs.tile([P, S], f32, tag="rsum", name="rsum")
                us = up.tile([P, S], bf16, tag=f"u{h}", name=f"u{h}")
                psu = [
                    ps_u.tile([P, QC], f32, tag="u", name=f"psu{c}")
                    for c in range(NQC)
                ]
                for kb in range(NKB):
                    c0 = kb // (QC // P)
                    doff = kb * P - c0 * QC
                    for c in range(c0, NQC):
                        psp = ps_w.tile([P, QC], f32, tag="w", name="psp")
                        nc.tensor.matmul(
                            psp,
                            k_sb[h][:, kb * P:(kb + 1) * P],
                            q_sb[h][:, c * QC:(c + 1) * QC],
                            start=True,
                            stop=True,
                        )
                        if c == c0:
                            nc.vector.tensor_add(
                                psp[:, doff:doff + P],
                                psp[:, doff:doff + P],
                                tril_sb,
                            )
                        p_t = pp.tile([P, QC], bf16, tag="p", name="p_t")
                        off = doff if c == c0 else 0
                        nc.scalar.activation(
                            p_t[:, off:], psp[:, off:], Act.Exp, scale=SCALE
                        )
                        if off > 0:
                            nc.vector.memset(p_t[:, :off], 0.0)
                        if kb == 0:
                            nc.vector.tensor_copy(
                                out=rsum[:, c * QC:(c + 1) * QC], in_=p_t
                            )
                        else:
                            nc.vector.tensor_add(
                                rsum[:, c * QC:(c + 1) * QC],
                                rsum[:, c * QC:(c + 1) * QC],
                                p_t,
                            )
                        nc.tensor.matmul(
                            psu[c],
                            v_sb[:, kb, h * DH:(h + 1) * DH],
                            p_t,
                            start=(kb == 0),
                            stop=(kb == (QC // P) * c + (QC // P) - 1),
                        )
                        if kb == (QC // P) * c + (QC // P) - 1:
                            nc.scalar.copy(us[:, c * QC:(c + 1) * QC], psu[c])
                # denominators: per q-block ones-matmul then reciprocal
                psr = ps_w.tile([P, NTB], f32, tag="w", name="psr")
                for qb in range(NTB):
                    nc.tensor.matmul(
                        psr[:, qb:qb + 1],
                        rsum[:, qb * P:(qb + 1) * P],
                        ones_sb,
                        start=True,
                        stop=True,
                    )
                ri = rs.tile([P, NTB], f32, tag=f"rinv{h}", name=f"rinv{h}")
                nc.vector.reciprocal(ri, psr)
                rinv.append(ri)
                u_sb.append(us)

            # ---------------- out projection ----------------
            for tb in range(NTB):
                for cc in range(D // QC):
                    ps0 = ps_w.tile([P, QC], f32, tag="w", name="ps0")
                    nc.tensor.matmul(
                        ps0,
                        u_sb[0][:, tb * P:(tb + 1) * P],
                        wo_sb[:, 0, cc * QC:(cc + 1) * QC],
                        start=True,
                        stop=True,
                    )
                    ps1 = ps_w.tile([P, QC], f32, tag="w", name="ps1")
                    nc.tensor.matmul(
                        ps1,
                        u_sb[1][:, tb * P:(tb + 1) * P],
                        wo_sb[:, 1, cc * QC:(cc + 1) * QC],
                        start=True,
                        stop=True,
                    )
                    f_t = fin.tile([P, QC], f32, tag="fin", name="f_t")
                    nc.scalar.activation(
                        f_t, ps0, Act.Copy, scale=rinv[0][:, tb:tb + 1]
                    )
                    o1 = op.tile([P, QC], f32, tag="o1", name="o1")
                    nc.scalar.activation(
                        o1, ps1, Act.Copy, scale=rinv[1][:, tb:tb + 1]
                    )
                    nc.vector.tensor_add(f_t, f_t, o1)
                    nc.vector.tensor_add(
                        f_t, f_t, brep_sb[:, cc * QC:(cc + 1) * QC]
                    )
                    nc.sync.dma_start(
                        out_p[b, tb * P:(tb + 1) * P, cc * QC:(cc + 1) * QC],
                        f_t,
                    )


_CACHE = {}


def _build():
    if "nc" in _CACHE:
        return _CACHE["nc"]
    nc = bacc.Bacc("TRN2", target_bir_lowering=False, debug=False)
    xty = nc.dram_tensor("xty", [B, D, S], bf16, kind="ExternalInput").ap()
    wqkv = nc.dram_tensor("wqkv", [D, 6 * P], bf16, kind="ExternalInput").ap()
    bqk = nc.dram_tensor("bqk", [P, 4], f32, kind="ExternalInput").ap()
    wout = nc.dram_tensor("wout", [HPC * DH, D], bf16, kind="ExternalInput").ap()
    brep = nc.dram_tensor("brep", [P, D], f32, kind="ExternalInput").ap()
    trilm = nc.dram_tensor("trilm", [P, P], f32, kind="ExternalInput").ap()
    ones = nc.dram_tensor("ones", [P, 1], f32, kind="ExternalInput").ap()
    out_p = nc.dram_tensor("out_p", [B, S, D], f32, kind="ExternalOutput").ap()
    with tile.TileContext(nc) as tctx:
        _emit(nc, tctx, (xty, wqkv, bqk, wout, brep, trilm, ones, out_p))
    nc.compile()
    _CACHE["nc"] = nc
    return nc


def _in_maps(x, W_qkv, b_qkv, W_out, b_out):
    import ml_dtypes

    bf = ml_dtypes.bfloat16
    trilm = np.where(
        np.arange(P)[None, :] >= np.arange(P)[:, None], 0.0, -1e9
    ).astype(np.float32)
    ones = np.ones((P, 1), dtype=np.float32)
    xty = np.ascontiguousarray(x.transpose(0, 2, 1)).astype(bf)
    maps = []
    for core in range(NCORES):
        h0 = core * HPC
        cols = []
        for off in (0, D, 2 * D):  # q, k, v column groups of W_qkv
            for h in range(h0, h0 + HPC):
                cols.append((off + h * DH, off + (h + 1) * DH))
        wqkv_c = np.concatenate(
            [W_qkv[:, a:b_] for a, b_ in cols], axis=1
        ).astype(bf)
        bqk_c = np.stack(
            [b_qkv[a:b_] for a, b_ in cols[:4]], axis=1
        ).astype(np.float32)  # [128, 4]
        bv_c = np.concatenate(
            [b_qkv[a:b_] for a, b_ in cols[4:]]
        ).astype(np.float32)  # [256]
        wout_c = W_out[h0 * DH:(h0 + HPC) * DH, :].astype(np.float32)
        bias_fold = (b_out / NCORES + bv_c @ wout_c).astype(np.float32)
        brep_c = np.broadcast_to(bias_fold, (P, D)).copy()
        maps.append({
            "xty": xty,
            "wqkv": np.ascontiguousarray(wqkv_c),
            "bqk": np.ascontiguousarray(bqk_c),
            "wout": np.ascontiguousarray(wout_c.astype(bf)),
            "brep": brep_c,
            "trilm": trilm,
            "ones": ones,
        })
    return maps


def kernel(x, W_qkv, b_qkv, W_out, b_out, _trace=False, _trace_kwargs=None):
    x = np.asarray(x, dtype=np.float32)
    W_qkv = np.asarray(W_qkv, dtype=np.float32)
    b_qkv = np.asarray(b_qkv, dtype=np.float32)
    W_out = np.asarray(W_out, dtype=np.float32)
    b_out = np.asarray(b_out, dtype=np.float32)

    nc = _build()
    maps = _in_maps(x, W_qkv, b_qkv, W_out, b_out)
    res = run_bass_kernel_spmd(
        nc, maps, core_ids=list(range(NCORES)), trace=_trace,
        **(_trace_kwargs or {}),
    )
    out = res.results[0]["out_p"]
    for c in range(1, NCORES):
        out = out + res.results[c]["out_p"]
    if _trace:
        _CACHE["last_results"] = res
    return out.astype(np.float32)

